# revision 10
# baseline (speedup 1.0000x reference)
"""Deformable-conv block (dense_cnn nn_DL_16638703305357) as a Bass/Tile kernel.

Data-parallel over batch: 8 samples -> 8 NeuronCores, one sample per core.

Per-core pipeline (bf16 data, fp32 PSUM):
  1. conv1x1 (w1|w2 fused, K=128) on PE -> SiLU on ACT -> x1 (pitch-130 SBUF
     copy for the 3x3 conv + flat DRAM copy), x2 (straight to DRAM stage).
  2. offset conv (3x3) = 9 accumulating PE matmuls over the pitch-130 x1.
  3. hat weight maps from PSUM: P=relu(-off-b), M=relu(off+b), Z=1-P-M
     (bilinear weights for window offsets -1/0/+1; exact since |off|<0.5).
     Staged to DRAM, expanded to 81 rows (tap x 3y x 3x) via replication
     DMAs, multiplied in one TT per block, edge-validity zeroed by memsets.
  4. x1 reshuffled into combine layout [128=(8 pos-blocks x 16 ch), 2048+halo]
     (4 channel-chunks); maps replicated x16 into the same layout by DMA.
  5. 81-term weighted-window accumulation (deformable bilinear sampling) as
     bf16 tensor_tensor mult/add passes on DVE + GPSIMD -> per-tap S~_k.
  6. S~ staged to DRAM in natural [tap*64+c, pos] layout; final 1x1 conv as 5
     K=128 pair-matmuls accumulating in PSUM; SiLU; fp32 out.
"""

import numpy as np

import concourse.bacc as bacc
import concourse.bass as bass
import concourse.tile as tile
from concourse import mybir
from concourse import bass_utils

B, CIN, COUT, H, W = 8, 128, 128, 128, 128
HID = 64
NPOS = H * W            # 16384
PITCH = W + 2           # row pitch of the conv-padded x1 copy
XPITCH = NPOS + 1024    # flat x1 dram pitch (512 halo each side)
EXT = 512
JBLK = 8
QLEN = NPOS // JBLK     # 2048
QHALO = 258             # max |window shift| = 2*128 + 2
QEXT = QLEN + 2 * QHALO

F32 = mybir.dt.float32
BF16 = mybir.dt.bfloat16
AF = mybir.ActivationFunctionType
OP = mybir.AluOpType

_CACHE = {}


def ap_of(t, ap, off=0):
    base = t[:, :] if not isinstance(t, bass.AP) else t
    return bass.AP(tensor=base.tensor, offset=base.offset + off, ap=ap)


def _build():
    nc = bacc.Bacc("TRN2", target_bir_lowering=False, debug=False, num_devices=8)

    xd = nc.dram_tensor("x", [CIN, NPOS], F32, kind="ExternalInput")
    w12T = nc.dram_tensor("w12T", [128, 128], BF16, kind="ExternalInput")
    b12 = nc.dram_tensor("b12", [128, 1], F32, kind="ExternalInput")
    offwT = nc.dram_tensor("offwT", [HID, 9 * 18], BF16, kind="ExternalInput")
    offbP = nc.dram_tensor("offbP", [18, 1], F32, kind="ExternalInput")
    offbN = nc.dram_tensor("offbN", [18, 1], F32, kind="ExternalInput")
    w3p = nc.dram_tensor("w3p", [128, 5 * 128], BF16, kind="ExternalInput")
    s3d = nc.dram_tensor("s3", [128, 1], F32, kind="ExternalInput")
    b3d = nc.dram_tensor("b3", [128, 1], F32, kind="ExternalInput")
    outd = nc.dram_tensor("out", [COUT, NPOS], F32, kind="ExternalOutput")

    NCH = 32
    CHK = NPOS // NCH   # 512 positions = 4 image rows per chunk

    with tile.TileContext(nc) as tc:
        with (
            tc.tile_pool(name="sing", bufs=1) as sing,
            tc.tile_pool(name="drm", bufs=1, space="DRAM") as drm,
        ):
            w12sb = sing.tile([128, 128], BF16, tag="w12")
            nc.gpsimd.dma_start(out=w12sb[:, :], in_=w12T.ap()[:, :])
            b12sb = sing.tile([128, 1], F32, tag="b12")
            nc.gpsimd.dma_start(out=b12sb[:, :], in_=b12.ap()[:, :])
            offwsb = sing.tile([HID, 9 * 18], BF16, tag="offw")
            nc.gpsimd.dma_start(out=offwsb[:, :], in_=offwT.ap()[:, :])
            obP = sing.tile([18, 1], F32, tag="obP")
            nc.gpsimd.dma_start(out=obP[:, :], in_=offbP.ap()[:, :])
            obN = sing.tile([18, 1], F32, tag="obN")
            nc.gpsimd.dma_start(out=obN[:, :], in_=offbN.ap()[:, :])
            w3sb = sing.tile([128, 5 * 128], BF16, tag="w3p")
            nc.gpsimd.dma_start(out=w3sb[:, :], in_=w3p.ap()[:, :])
            s3sb = sing.tile([128, 1], F32, tag="s3")
            nc.gpsimd.dma_start(out=s3sb[:, :], in_=s3d.ap()[:, :])
            b3sb = sing.tile([128, 1], F32, tag="b3")
            nc.gpsimd.dma_start(out=b3sb[:, :], in_=b3d.ap()[:, :])

            amap = sing.tile([81, NPOS], BF16, tag="amap")
            x1r = [sing.tile([128, QEXT], BF16, tag=f"x1r{c}", name=f"x1r{c}")
                   for c in range(4)]

            x1fd = drm.tile([HID, XPITCH], BF16, tag="x1fd")      # flat + halo
            famd = drm.tile([54, NPOS], BF16, tag="famd")         # P | Z | M
            sdram = drm.tile([640, NPOS], BF16, tag="sdram")
            amapd = drm.tile([81, NPOS], BF16, tag="amapd")

            # zero halos of flat x1
            zt = sing.tile([HID, EXT], BF16, tag="zt")
            nc.vector.memset(zt[:, :], 0.0)
            nc.gpsimd.dma_start(out=x1fd[:, 0:EXT], in_=zt[:, :])
            nc.gpsimd.dma_start(out=x1fd[:, EXT + NPOS:XPITCH], in_=zt[:, :])

            with (
                tc.tile_pool(name="ph1", bufs=3) as ph1,
                tc.tile_pool(name="x1cp", bufs=1) as x1cp,
                tc.tile_pool(name="mape", bufs=2) as mape,
                tc.tile_pool(name="ps1", bufs=2, space="PSUM") as ps1,
                tc.tile_pool(name="pso", bufs=2, space="PSUM") as pso,
            ):
                x1c = x1cp.tile([HID, PITCH * (H + 2)], BF16, tag="x1c")
                nc.vector.memset(x1c[:, :], 0.0)

                # ---- conv1: x -> x1, x2 ----
                for ci in range(8):
                    xbf = ph1.tile([CIN, 2048], BF16, tag="xbf")
                    nc.gpsimd.dma_start(
                        out=xbf[:, :], in_=xd.ap()[:, ci * 2048:(ci + 1) * 2048]
                    )
                    for s in range(4):
                        n0 = ci * 2048 + s * 512
                        y0 = n0 // W
                        ps = ps1.tile([128, 512], F32, tag="c1")
                        nc.tensor.matmul(
                            ps[:, :], w12sb[:, :], xbf[:, s * 512:(s + 1) * 512],
                            start=True, stop=True,
                        )
                        x1stg = ph1.tile([HID, 512], BF16, tag="x1stg")
                        nc.scalar.activation(
                            x1stg[:, :], ps[0:HID, :], AF.Silu,
                            bias=b12sb[0:HID, :], scale=1.0,
                        )
                        nc.gpsimd.dma_start(
                            out=x1fd[:, EXT + n0:EXT + n0 + 512],
                            in_=x1stg[:, :],
                        )
                        nc.scalar.activation(
                            ap_of(x1c, [[PITCH * (H + 2), HID], [PITCH, 4],
                                        [1, W]],
                                  off=PITCH * (1 + y0) + 1),
                            ps[0:HID, :], AF.Silu,
                            bias=b12sb[0:HID, :], scale=1.0,
                        )
                        x2stg = ph1.tile([128, 512], BF16, tag="x2stg")
                        nc.scalar.activation(
                            x2stg[HID:128, :], ps[HID:128, :], AF.Silu,
                            bias=b12sb[HID:128, :], scale=1.0,
                        )
                        nc.gpsimd.dma_start(
                            out=ap_of(sdram[:, :], [[NPOS, HID], [1, 512]],
                                      off=576 * NPOS + n0),
                            in_=x2stg[HID:128, :],
                        )

                # ---- offset conv + hat families ----
                for ch in range(NCH):
                    n0 = ch * CHK
                    y0 = n0 // W
                    po = pso.tile([18, CHK], F32, tag="off")
                    for t in range(9):
                        ky, kx = t // 3, t % 3
                        rhs = ap_of(
                            x1c, [[PITCH * (H + 2), HID], [PITCH, 4], [1, W]],
                            off=PITCH * (y0 + ky) + kx,
                        )
                        nc.tensor.matmul(
                            po[:, :], offwsb[:, t * 18:(t + 1) * 18], rhs,
                            start=(t == 0), stop=(t == 8),
                        )
                    pstg = ph1.tile([18, CHK], BF16, tag="pstg")
                    mstg = ph1.tile([18, CHK], BF16, tag="mstg")
                    zstg = ph1.tile([18, CHK], BF16, tag="zstg")
                    tstg = ph1.tile([18, CHK], BF16, tag="tstg")
                    nc.scalar.activation(pstg[:, :], po[:, :], AF.Relu,
                                         bias=obN[:, :], scale=-1.0)
                    nc.scalar.activation(mstg[:, :], po[:, :], AF.Relu,
                                         bias=obP[:, :], scale=1.0)
                    nc.gpsimd.tensor_tensor(tstg[:, :], pstg[:, :], mstg[:, :],
                                            OP.add)
                    nc.vector.tensor_scalar(zstg[:, :], tstg[:, :],
                                            -1.0, 1.0, OP.mult, OP.add)
                    nc.gpsimd.dma_start(out=famd[0:18, n0:n0 + CHK],
                                        in_=pstg[:, :])
                    nc.gpsimd.dma_start(out=famd[18:36, n0:n0 + CHK],
                                        in_=zstg[:, :])
                    nc.gpsimd.dma_start(out=famd[36:54, n0:n0 + CHK],
                                        in_=mstg[:, :])

                # ---- edge-validity zeroing (on DRAM-staged families) ----
                # y-edges: zero family-a dy-rows (2k) where y+ky-1+(a-1) OOB
                for ky in range(3):
                    for a in range(3):
                        s = (ky - 1) + (a - 1)
                        if s == 0:
                            continue
                        nbad = abs(s) * W
                        f0 = 0 if s < 0 else NPOS - nbad
                        nc.gpsimd.dma_start(
                            out=ap_of(famd[:, :], [[2 * NPOS, 3], [1, nbad]],
                                      off=(a * 18 + 2 * ky * 3) * NPOS + f0),
                            in_=ap_of(zt, [[EXT, 3], [1, nbad]]),
                        )
                # x-edges: zero family-b dx-rows (2k+1) where x+kx-1+(b-1) OOB
                for kx in range(3):
                    for b in range(3):
                        s = (kx - 1) + (b - 1)
                        if s == 0:
                            continue
                        f0 = 0 if s < 0 else W - abs(s)
                        nc.gpsimd.dma_start(
                            out=ap_of(famd[:, :],
                                      [[6 * NPOS, 3], [W, H], [1, abs(s)]],
                                      off=(b * 18 + 2 * kx + 1) * NPOS + f0),
                            in_=ap_of(zt, [[EXT, 3], [abs(s), H], [1, abs(s)]]),
                        )

                # ---- expand families to 81 rows, product per j-block ----
                # row order: r = a*27 + t*3 + b  (a=y-window, b=x-window)
                for j in range(JBLK):
                    q0 = j * QLEN
                    cyE = mape.tile([81, QLEN], BF16, tag="cyE")
                    cxE = mape.tile([81, QLEN], BF16, tag="cxE")
                    for a in range(3):   # rows [a*27, (a+1)*27) <- famA[2k]
                        nc.gpsimd.dma_start(
                            out=cyE[a * 27:(a + 1) * 27, :],
                            in_=ap_of(famd[:, :],
                                      [[2 * NPOS, 9], [0, 3], [1, QLEN]],
                                      off=a * 18 * NPOS + q0),
                        )
                    for b in range(3):   # rows 3m+b <- famB[2(m%9)+1]
                        nc.gpsimd.dma_start(
                            out=ap_of(cxE, [[3 * QLEN, 27], [1, QLEN]],
                                      off=b * QLEN),
                            in_=ap_of(famd[:, :],
                                      [[0, 3], [2 * NPOS, 9], [1, QLEN]],
                                      off=(b * 18 + 1) * NPOS + q0),
                        )
                    nc.vector.tensor_tensor(
                        amap[:, q0:q0 + QLEN], cyE[:, :], cxE[:, :], OP.mult
                    )
                nc.gpsimd.dma_start(out=amapd[:, :], in_=amap[:, :])

                # ---- x1 -> combine layout ----
                for cb in range(4):
                    nc.gpsimd.dma_start(
                        out=x1r[cb][:, :],
                        in_=ap_of(x1fd[:, :],
                                  [[QLEN, JBLK], [XPITCH, 16], [1, QEXT]],
                                  off=cb * 16 * XPITCH + EXT - QHALO),
                    )

            # ================= combine + final =================
            with (
                tc.tile_pool(name="wr", bufs=10) as wrp,
                tc.tile_pool(name="cmb", bufs=3) as cmb,
                tc.tile_pool(name="stg", bufs=5) as stg,
                tc.tile_pool(name="frh", bufs=4) as frh,
                tc.tile_pool(name="psf", bufs=2, space="PSUM") as psf,
                tc.tile_pool(name="oute", bufs=3) as outp,
            ):
                for t in range(9):
                    ky, kx = t // 3, t % 3
                    wrt = []
                    for r in range(9):
                        a, b = r // 3, r % 3
                        row = a * 27 + t * 3 + b
                        wt = wrp.tile([128, QLEN], BF16, tag="wr", name="wt")
                        nc.gpsimd.dma_start(
                            out=wt[:, :],
                            in_=ap_of(amapd[:, :],
                                      [[QLEN, JBLK], [0, 16], [1, QLEN]],
                                      off=row * NPOS),
                        )
                        wrt.append(wt)
                    for cb in range(4):
                        eng = nc.gpsimd if (cb == 3 and t < 8) else nc.vector
                        acc = stg.tile([128, QLEN], BF16, tag="acc")
                        for r in range(9):
                            a, b = r // 3, r % 3
                            S = (ky - 2 + a) * W + (kx - 2 + b)
                            src = x1r[cb][:, QHALO + S:QHALO + S + QLEN]
                            if r == 0:
                                eng.tensor_tensor(acc[:, :], wrt[r][:, :], src,
                                                  OP.mult)
                            else:
                                tmp = cmb.tile([128, QLEN], BF16, tag="tmp")
                                eng.tensor_tensor(tmp[:, :], wrt[r][:, :], src,
                                                  OP.mult)
                                eng.tensor_tensor(acc[:, :], acc[:, :],
                                                  tmp[:, :], OP.add)
                        nc.gpsimd.dma_start(
                            out=ap_of(sdram[:, :],
                                      [[QLEN, JBLK], [NPOS, 16], [1, QLEN]],
                                      off=(t * 64 + cb * 16) * NPOS),
                            in_=acc[:, :],
                        )

                # ---- final conv ----
                for n in range(32):
                    pf = psf.tile([128, 512], F32, tag="fin")
                    for p in range(5):
                        rh = frh.tile([128, 512], BF16, tag="frh")
                        nc.gpsimd.dma_start(
                            out=rh[:, :],
                            in_=sdram[p * 128:(p + 1) * 128,
                                      n * 512:(n + 1) * 512],
                        )
                        nc.tensor.matmul(
                            pf[:, :], w3sb[:, p * 128:(p + 1) * 128], rh[:, :],
                            start=(p == 0), stop=(p == 4),
                        )
                    ot = outp.tile([128, 512], F32, tag="oute")
                    nc.scalar.activation(ot[:, :], pf[:, :], AF.Silu,
                                         bias=b3sb[:, :], scale=s3sb[:, :])
                    nc.gpsimd.dma_start(
                        out=outd.ap()[:, n * 512:(n + 1) * 512], in_=ot[:, :]
                    )

    nc.compile()
    return nc


def _consts(w1, s1, b1, w2, s2, b2, w3, s3, b3, off_w, off_b, dw):
    c = {}
    W12 = np.concatenate([w1 * s1[:, None], w2 * s2[:, None]], axis=0)
    c["w12T"] = W12.T.astype(np.float32)
    c["b12"] = np.concatenate([b1, b2])[:, None].astype(np.float32)
    owT = np.zeros((HID, 9 * 18), np.float32)
    for t in range(9):
        ky, kx = t // 3, t % 3
        owT[:, t * 18:(t + 1) * 18] = off_w[:, :, ky, kx].T
    c["offwT"] = owT
    c["offbP"] = off_b[:, None].astype(np.float32)
    c["offbN"] = (-off_b)[:, None].astype(np.float32)
    w3a, w3b = w3[:, :HID], w3[:, HID:]
    dwf = dw.reshape(HID, 9)
    pairs = np.zeros((128, 5 * 128), np.float32)
    for p in range(4):
        ta, tb = 2 * p, 2 * p + 1
        pairs[0:64, p * 128:(p + 1) * 128] = (w3a * dwf[:, ta][None, :]).T
        pairs[64:128, p * 128:(p + 1) * 128] = (w3a * dwf[:, tb][None, :]).T
    pairs[0:64, 512:640] = (w3a * dwf[:, 8][None, :]).T
    pairs[64:128, 512:640] = w3b.T
    c["w3p"] = pairs
    c["s3"] = s3[:, None].astype(np.float32)
    c["b3"] = b3[:, None].astype(np.float32)
    return c


def _bf16(a):
    import ml_dtypes
    return a.astype(ml_dtypes.bfloat16)


def kernel(**inputs):
    x = np.asarray(inputs["x"], np.float32)
    consts = _consts(
        *(np.asarray(inputs[k], np.float32)
          for k in ["w1", "s1", "b1", "w2", "s2", "b2", "w3", "s3", "b3",
                    "off_w", "off_b", "dw"])
    )
    if "nc" not in _CACHE:
        _CACHE["nc"] = _build()
    nc = _CACHE["nc"]

    cmap = {k: (_bf16(v) if k in ("w12T", "offwT", "w3p") else v)
            for k, v in consts.items()}
    in_maps = []
    for b in range(B):
        m = dict(cmap)
        m["x"] = x[b].reshape(CIN, NPOS)
        in_maps.append(m)
    res = bass_utils.run_bass_kernel_spmd(nc, in_maps, core_ids=list(range(B)))
    out = np.stack([res.results[b]["out"].astype(np.float32) for b in range(B)])
    return out.reshape(B, COUT, H, W)
